# revision 12
# baseline (speedup 1.0000x reference)
"""2-layer GCN (200k nodes, 512->16->7, 6.4M random edges + self-loops) on
8 Trainium2 NeuronCores.

Strategy (graph/data parallel, dst-sharded):
  - 25000 dst nodes per core. Host pre-sorts edges by (dst-core, chunk,
    src-shard, dst) and builds int16 gather/extraction index streams.
  - Layer linear: per-core matmul of its x^T slice with W1 (fp32 on PE),
    prescaled by dis[src]; AllGather the tiny [16, 25000] hidden table so
    every core holds h' for all 200k src nodes in SBUF ([128, 25008]:
    q7-core q's 16 partitions = feature rows of src shard q).
  - Edge aggregation: GPSIMD ap_gather streams h'[src] feature-columns for
    dst-sorted edges (8 q7 cores in parallel over the 8 src shards), DVE
    tensor_tensor_scan computes a running cumsum per partition, a second
    ap_gather extracts the cumsum at per-node segment boundaries, and the
    shifted difference gives per-(src-shard, feature, dst) partial sums.
    A 0/1 block-selector matmul on PE sums the 8 shard partials.
  - Postscale by dis[dst] (+bias/relu), second layer identical with the
    16->7 linear fused before the AllGather, then log_softmax in the
    [vb-block, feature] layout and one output DMA per core.

kernel() accepts FULL inputs and returns the FULL [200000, 7] output.
"""

import numpy as np

N = 200000
C = 8            # cores
S = N // C       # dst nodes per core
VBS = S // 8     # 3125, nodes per vb partition-group
NVB = 8
CH = 512         # edge-chunk node width
CW = [CH] * 6 + [VBS - 6 * CH]   # widths per vb: 6x512 + 53
TPV = len(CW)    # chunks per vb
T = NVB * TPV    # 56 chunks per core
ZIDX = S         # zero-row index in the gather table
NELEM = S + 8    # table columns (25000 real + 8 zero pad)

_cache = {}


# ----------------------------------------------------------------------------
# host preprocessing
# ----------------------------------------------------------------------------

def _preprocess(edge_index):
    src = np.concatenate([np.asarray(edge_index[0]), np.arange(N, dtype=np.int64)])
    dst = np.concatenate([np.asarray(edge_index[1]), np.arange(N, dtype=np.int64)])
    deg = np.bincount(dst, minlength=N).astype(np.float64)
    dis = (1.0 / np.sqrt(deg)).astype(np.float32)

    ci = dst // S
    vlocal = (dst % S).astype(np.int64)
    vb = vlocal // VBS
    w = vlocal % VBS
    cc = np.minimum(w // CH, TPV - 1)
    t = vb * TPV + cc
    q = src // S
    slocal = (src % S).astype(np.int16)

    tg = (ci * T + t) * 8 + q                     # global group id [0, C*T*8)
    order = np.argsort(tg * np.int64(S + 1) + vlocal, kind="stable")
    tg_s = tg[order]
    sl_s = slocal[order]
    vl_s = vlocal[order]

    counts = np.bincount(tg, minlength=C * T * 8)
    grp_off = np.zeros(C * T * 8 + 1, dtype=np.int64)
    np.cumsum(counts, out=grp_off[1:])
    counts_ct = counts.reshape(C, T, 8)

    # per-chunk stream width (uniform across cores/q7s: SPMD shared program).
    # Chunk index slices must start 64B-aligned in the resident idx tiles
    # (the q7 ucode reads idx vectors with wide aligned loads), so place
    # each chunk's columns at a 32-column boundary.
    ALN = 32
    Wt = []
    for tt in range(T):
        m = int(counts_ct[:, tt, :].max())
        Wt.append(int(np.ceil((m + 1) / 16) * 16))
    off16 = np.zeros(T + 1, dtype=np.int64)
    for tt in range(T):
        off16[tt + 1] = -(-(off16[tt] + Wt[tt] // 16) // ALN) * ALN
    GW = int(off16[-1])

    nw_t = [CW[tt % TPV] for tt in range(T)]
    Et = [int(np.ceil((nw + 1) / 16) * 16) for nw in nw_t]
    eoff16 = np.zeros(T + 1, dtype=np.int64)
    for tt in range(T):
        eoff16[tt + 1] = -(-(eoff16[tt] + Et[tt] // 16) // ALN) * ALN
    EW = int(eoff16[-1])

    # gather index streams: pos 0 = dummy zero-row, edges at 1..n, pad zero-row
    gidx = np.full((C, 128, GW), ZIDX, dtype=np.int16)
    pos = np.arange(len(tg_s), dtype=np.int64) - grp_off[tg_s] + 1
    e_ci = tg_s // (T * 8)
    rem = tg_s % (T * 8)
    e_tt = rem // 8
    e_qq = rem % 8
    gidx[e_ci, 16 * e_qq + (pos % 16), off16[e_tt] + pos // 16] = sl_s

    # extraction indices: [0] + inclusive prefix count per node in chunk
    eidx = np.zeros((C, 128, EW), dtype=np.int16)
    for cix in range(C):
        for tt in range(T):
            nw = nw_t[tt]
            v0 = (tt // TPV) * VBS + (tt % TPV) * CH
            base = (cix * T + tt) * 8
            for qq in range(8):
                g0 = grp_off[base + qq]
                n = counts[base + qq]
                jj = vl_s[g0:g0 + n] - v0
                ends = np.searchsorted(jj, np.arange(nw), side="right")
                ext = np.zeros(Et[tt], dtype=np.int16)
                ext[1:nw + 1] = ends
                k = np.arange(Et[tt])
                eidx[cix, 16 * qq + (k % 16), eoff16[tt] + k // 16] = ext

    return dis, gidx, eidx, Wt, off16, Et, eoff16, nw_t, GW, EW


def _build_consts(W1, b1, W2, b2):
    W1 = np.asarray(W1, np.float32)
    W2 = np.asarray(W2, np.float32)
    w1pad = np.zeros((NVB, 4, 128, 128), np.float32)
    for v in range(NVB):
        for kc in range(4):
            w1pad[v, kc, :, 16 * v:16 * v + 16] = W1[128 * kc:128 * kc + 128, :]
    selpad = np.zeros((NVB, 128, 128), np.float32)
    for v in range(NVB):
        for p in range(128):
            selpad[v, p, 16 * v + (p % 16)] = 1.0
    w2bd = np.zeros((128, 128), np.float32)
    for v in range(NVB):
        w2bd[16 * v:16 * v + 16, 16 * v:16 * v + 7] = W2
    blockones = np.zeros((128, 8), np.float32)
    for p in range(128):
        if p % 16 < 7:
            blockones[p, p // 16] = 1.0
    bcast8 = np.zeros((8, 128), np.float32)
    for p in range(128):
        bcast8[p // 16, p] = 1.0
    b1rep = np.asarray(b1, np.float32)[np.arange(128) % 16].reshape(128, 1)
    b2r = np.zeros(16, np.float32)
    b2r[:7] = np.asarray(b2, np.float32)
    b2rep = b2r[np.arange(128) % 16].reshape(128, 1)
    return w1pad, selpad, w2bd, blockones, bcast8, b1rep, b2rep


# ----------------------------------------------------------------------------
# device program
# ----------------------------------------------------------------------------

def _build_program(Wt, off16, Et, eoff16, nw_t, GW, EW, debug=False):
    import concourse.bacc as bacc
    import concourse.tile as tile
    import concourse.mybir as mybir
    from concourse import library_config

    dt = mybir.dt
    AF = mybir.ActivationFunctionType
    OP = mybir.AluOpType

    nc = bacc.Bacc("TRN2", target_bir_lowering=False, debug=False, num_devices=C)

    xTv_d = nc.dram_tensor("xTv", [NVB, 4, 128, VBS], dt.float32, kind="ExternalInput")
    disS_d = nc.dram_tensor("disS", [128, VBS], dt.float32, kind="ExternalInput")
    w1pad_d = nc.dram_tensor("w1pad", [NVB * 4, 128, 128], dt.float32, kind="ExternalInput")
    selpad_d = nc.dram_tensor("selpad", [NVB, 128, 128], dt.float32, kind="ExternalInput")
    w2bd_d = nc.dram_tensor("w2bd", [128, 128], dt.float32, kind="ExternalInput")
    bones_d = nc.dram_tensor("bones", [128, 8], dt.float32, kind="ExternalInput")
    bcast8_d = nc.dram_tensor("bcast8", [8, 128], dt.float32, kind="ExternalInput")
    b1rep_d = nc.dram_tensor("b1rep", [128, 1], dt.float32, kind="ExternalInput")
    b2rep_d = nc.dram_tensor("b2rep", [128, 1], dt.float32, kind="ExternalInput")
    gidx_d = nc.dram_tensor("gidx", [128, GW], dt.int16, kind="ExternalInput")
    eidx_d = nc.dram_tensor("eidx", [128, EW], dt.int16, kind="ExternalInput")
    outF_d = nc.dram_tensor("outF", [128, VBS], dt.float32, kind="ExternalOutput")
    if debug:
        dbg_h1pd = nc.dram_tensor("dbg_h1pd", [128, VBS], dt.float32, kind="ExternalOutput")
        dbg_table = nc.dram_tensor("dbg_table", [128, NELEM], dt.float32, kind="ExternalOutput")
        dbg_out1 = nc.dram_tensor("dbg_out1", [128, VBS], dt.float32, kind="ExternalOutput")
        dbg_out1p = nc.dram_tensor("dbg_out1p", [128, VBS], dt.float32, kind="ExternalOutput")
        dbg_h2pd = nc.dram_tensor("dbg_h2pd", [128, VBS], dt.float32, kind="ExternalOutput")
        dbg_table2 = nc.dram_tensor("dbg_table2", [128, NELEM], dt.float32, kind="ExternalOutput")
        dbg_out2 = nc.dram_tensor("dbg_out2", [128, VBS], dt.float32, kind="ExternalOutput")
        dbg_g0 = nc.dram_tensor("dbg_g0", [128, Wt[0]], dt.float32, kind="ExternalOutput")
        dbg_p0 = nc.dram_tensor("dbg_p0", [128, Et[0]], dt.float32, kind="ExternalOutput")
        dbg_d0 = nc.dram_tensor("dbg_d0", [128, CH], dt.float32, kind="ExternalOutput")

    with tile.TileContext(nc) as tc:
        nc.gpsimd.load_library(library_config.ap_gather)
        with (
            tc.tile_pool(name="consts", bufs=1) as consts,
            tc.tile_pool(name="state", bufs=3) as state,
            tc.tile_pool(name="psum", bufs=4, space="PSUM") as psum,
            tc.tile_pool(name="dram", bufs=1, space="DRAM") as dram,
        ):
            disS = consts.tile([128, VBS], dt.float32)
            nc.sync.dma_start(out=disS[:], in_=disS_d[:])
            selpad = consts.tile([128, NVB * 128], dt.float32)
            nc.sync.dma_start(
                out=selpad[:].rearrange("p (v m) -> p v m", m=128),
                in_=selpad_d.ap().rearrange("v p m -> p v m"),
            )
            w2bd = consts.tile([128, 128], dt.float32)
            nc.sync.dma_start(out=w2bd[:], in_=w2bd_d[:])
            bones = consts.tile([128, 8], dt.float32)
            nc.sync.dma_start(out=bones[:], in_=bones_d[:])
            bcast8 = consts.tile([8, 128], dt.float32)
            nc.sync.dma_start(out=bcast8[:], in_=bcast8_d[:])
            b1rep = consts.tile([128, 1], dt.float32)
            nc.sync.dma_start(out=b1rep[:], in_=b1rep_d[:])
            b2rep = consts.tile([128, 1], dt.float32)
            nc.sync.dma_start(out=b2rep[:], in_=b2rep_d[:])
            gidx_sb = consts.tile([128, GW], dt.int16)
            nc.sync.dma_start(out=gidx_sb[:], in_=gidx_d[:])
            eidx_sb = consts.tile([128, EW], dt.int16)
            nc.sync.dma_start(out=eidx_sb[:], in_=eidx_d[:])

            h1pd = state.tile([128, VBS], dt.float32, tag="state")

            # ---------------- phase X: h1' = (x @ W1) * dis[src] -------------
            with (
                tc.tile_pool(name="xt", bufs=8) as xtp,
                tc.tile_pool(name="w1p", bufs=1) as w1pp,
            ):
                w1pad = w1pp.tile([128, NVB * 4 * 128], dt.float32)
                nc.sync.dma_start(
                    out=w1pad[:].rearrange("p (v m) -> p v m", m=128),
                    in_=w1pad_d.ap().rearrange("v p m -> p v m"),
                )
                for u in range(NVB // 2):
                    xt = {}
                    for vi in range(2):
                        v = 2 * u + vi
                        for kc in range(4):
                            x1 = xtp.tile([128, VBS], dt.float32, tag="xt")
                            nc.sync.dma_start(out=x1[:], in_=xTv_d[v, kc].opt())
                            xt[(vi, kc)] = x1
                    for ccj in range(TPV):
                        wcc = CW[ccj]
                        c0 = ccj * CH
                        ps = psum.tile([128, CH], dt.float32, tag="ps")
                        for vi in range(2):
                            v = 2 * u + vi
                            for kc in range(4):
                                nc.tensor.matmul(
                                    ps[:, :wcc],
                                    lhsT=w1pad[:, (v * 4 + kc) * 128:(v * 4 + kc) * 128 + 128],
                                    rhs=xt[(vi, kc)][:, c0:c0 + wcc],
                                    start=(vi == 0 and kc == 0),
                                    stop=(vi == 1 and kc == 3),
                                )
                        nc.vector.tensor_mul(
                            out=h1pd[32 * u:32 * u + 32, c0:c0 + wcc],
                            in0=ps[32 * u:32 * u + 32, :wcc],
                            in1=disS[32 * u:32 * u + 32, c0:c0 + wcc],
                        )

            with tc.tile_pool(name="edge", bufs=1) as ep, \
                 tc.tile_pool(name="gbuf", bufs=2) as gp, \
                 tc.tile_pool(name="pbuf", bufs=2) as pp, \
                 tc.tile_pool(name="dbuf", bufs=2) as dp:
                table = ep.tile([128, NELEM], dt.float32)
                nc.vector.memset(table[:, S:NELEM], 0.0)

                def allgather(src_tile):
                    agin = dram.tile([128, VBS], dt.float32)
                    agout = dram.tile([C * 128, VBS], dt.float32, addr_space="Shared")
                    nc.sync.dma_start(out=agin[:], in_=src_tile[:])
                    nc.gpsimd.collective_compute(
                        "AllGather",
                        mybir.AluOpType.bypass,
                        replica_groups=[list(range(C))],
                        ins=[agin.opt()],
                        outs=[agout.opt()],
                    )
                    for qq in range(C):
                        blk = agout[128 * qq:128 * qq + 128, :]
                        nc.sync.dma_start(
                            out=table[16 * qq:16 * qq + 16, 0:S].rearrange(
                                "j (v w) -> j v w", v=NVB
                            ),
                            in_=blk.rearrange("(v j) w -> j v w", j=16),
                        )

                allgather(h1pd)
                if debug:
                    nc.sync.dma_start(out=dbg_h1pd[:], in_=h1pd[:])
                    nc.sync.dma_start(out=dbg_table[:], in_=table[:])

                _dbg_done = [False]

                def edge_chunk(tt):
                    """gather -> cumsum -> boundary extract -> shifted diff"""
                    nw = nw_t[tt]
                    wt = Wt[tt]
                    et = Et[tt]
                    g = gp.tile([128, wt], dt.float32, tag="g")
                    nc.gpsimd.ap_gather(
                        out_ap=g[:],
                        in_ap=table[:],
                        idxs_ap=gidx_sb[:, off16[tt]:off16[tt] + wt // 16],
                        channels=128, num_elems=NELEM, d=1, num_idxs=wt,
                    )
                    nc.vector.tensor_tensor_scan(
                        out=g[:], data0=g[:], data1=g[:], initial=0.0,
                        op0=OP.add, op1=OP.bypass,
                    )
                    p = pp.tile([128, et], dt.float32, tag="p")
                    nc.gpsimd.ap_gather(
                        out_ap=p[:],
                        in_ap=g[:],
                        idxs_ap=eidx_sb[:, eoff16[tt]:eoff16[tt] + et // 16],
                        channels=128, num_elems=wt, d=1, num_idxs=et,
                    )
                    d_ = dp.tile([128, CH], dt.float32, tag="d")
                    nc.any.tensor_tensor(
                        out=d_[:, :nw], in0=p[:, 1:nw + 1], in1=p[:, 0:nw],
                        op=OP.subtract,
                    )
                    if debug and tt == 0 and not _dbg_done[0]:
                        _dbg_done[0] = True
                        nc.sync.dma_start(out=dbg_g0[:], in_=g[:])
                        nc.sync.dma_start(out=dbg_p0[:], in_=p[:])
                        nc.sync.dma_start(out=dbg_d0[:], in_=d_[:])
                    return d_

                def edge_layer(out_t):
                    # vb pairs: all partition-sliced ops are [32] @ 32-aligned
                    for u in range(NVB // 2):
                        for ccj in range(TPV):
                            nw = CW[ccj]
                            c0 = ccj * CH
                            ps = psum.tile([128, CH], dt.float32, tag="ps")
                            for vi in range(2):
                                v = 2 * u + vi
                                d_ = edge_chunk(v * TPV + ccj)
                                nc.tensor.matmul(
                                    ps[:, :nw],
                                    lhsT=selpad[:, 128 * v:128 * v + 128],
                                    rhs=d_[:, :nw],
                                    start=(vi == 0), stop=(vi == 1),
                                )
                            nc.scalar.activation(
                                out_t[32 * u:32 * u + 32, c0:c0 + nw],
                                ps[32 * u:32 * u + 32, :nw],
                                AF.Copy,
                            )

                # ---------------- layer 1 aggregation -----------------------
                out1 = state.tile([128, VBS], dt.float32, tag="state")
                edge_layer(out1)
                if debug:
                    nc.sync.dma_start(out=dbg_out1[:], in_=out1[:])
                nc.vector.tensor_mul(out=out1[:], in0=out1[:], in1=disS[:])
                nc.scalar.activation(out1[:], out1[:], AF.Relu, bias=b1rep[:])
                nc.vector.tensor_mul(out=out1[:], in0=out1[:], in1=disS[:])
                if debug:
                    nc.sync.dma_start(out=dbg_out1p[:], in_=out1[:])

                # ---------------- layer 2 linear + table --------------------
                h2pd = state.tile([128, VBS], dt.float32, tag="state")
                for ccj in range(TPV):
                    wcc = CW[ccj]
                    c0 = ccj * CH
                    ps = psum.tile([128, CH], dt.float32, tag="ps")
                    nc.tensor.matmul(
                        ps[:, :wcc], lhsT=w2bd[:], rhs=out1[:, c0:c0 + wcc],
                        start=True, stop=True,
                    )
                    nc.scalar.activation(h2pd[:, c0:c0 + wcc], ps[:, :wcc], AF.Copy)
                allgather(h2pd)
                if debug:
                    nc.sync.dma_start(out=dbg_h2pd[:], in_=h2pd[:])
                    nc.sync.dma_start(out=dbg_table2[:], in_=table[:])

                # ---------------- layer 2 aggregation -----------------------
                out2 = state.tile([128, VBS], dt.float32, tag="state")
                edge_layer(out2)
                nc.vector.tensor_mul(out=out2[:], in0=out2[:], in1=disS[:])
                nc.vector.tensor_scalar_add(out2[:], out2[:], b2rep[:])
                if debug:
                    nc.sync.dma_start(out=dbg_out2[:], in_=out2[:])

            # ---------------- log_softmax over the 7 classes ----------------
            with tc.tile_pool(name="fin", bufs=1) as fp:
                expb = fp.tile([128, VBS], dt.float32)
                nc.scalar.activation(expb[:], out2[:], AF.Exp)
                z8 = fp.tile([8, VBS], dt.float32)
                for ccj in range(TPV):
                    wcc = CW[ccj]
                    c0 = ccj * CH
                    psz = psum.tile([8, CH], dt.float32, tag="psz")
                    nc.tensor.matmul(
                        psz[:, :wcc], lhsT=bones[:], rhs=expb[:, c0:c0 + wcc],
                        start=True, stop=True,
                    )
                    nc.scalar.activation(z8[:, c0:c0 + wcc], psz[:, :wcc], AF.Copy)
                l8 = fp.tile([8, VBS], dt.float32)
                nc.scalar.activation(l8[:], z8[:], AF.Ln)
                outF = state.tile([128, VBS], dt.float32, tag="state")
                for ccj in range(TPV):
                    wcc = CW[ccj]
                    c0 = ccj * CH
                    psb = psum.tile([128, CH], dt.float32, tag="ps")
                    nc.tensor.matmul(
                        psb[:, :wcc], lhsT=bcast8[:], rhs=l8[:, c0:c0 + wcc],
                        start=True, stop=True,
                    )
                    nc.vector.tensor_tensor(
                        out=outF[:, c0:c0 + wcc], in0=out2[:, c0:c0 + wcc],
                        in1=psb[:, :wcc], op=OP.subtract,
                    )
                nc.sync.dma_start(out=outF_d[:], in_=outF[:])

    nc.compile()
    return nc


# ----------------------------------------------------------------------------
# entry point
# ----------------------------------------------------------------------------

def kernel(x, edge_index, W1, b1, W2, b2, _trace=False, _debug=False):
    import concourse.bass_utils as bass_utils

    x = np.asarray(x, np.float32)
    edge_index = np.asarray(edge_index)

    dis, gidx, eidx, Wt, off16, Et, eoff16, nw_t, GW, EW = _preprocess(edge_index)
    w1pad, selpad, w2bd, bones, bcast8, b1rep, b2rep = _build_consts(W1, b1, W2, b2)

    key = ("prog", tuple(Wt), GW, EW, _debug)
    if key not in _cache:
        _cache[key] = _build_program(Wt, off16, Et, eoff16, nw_t, GW, EW, debug=_debug)
    nc = _cache[key]

    in_maps = []
    for c in range(C):
        xs = np.ascontiguousarray(x[c * S:(c + 1) * S].T)      # [512, S]
        xTv = np.empty((NVB, 4, 128, VBS), np.float32)
        for v in range(NVB):
            for kc in range(4):
                xTv[v, kc] = xs[128 * kc:128 * kc + 128, VBS * v:VBS * v + VBS]
        disS = np.repeat(
            dis[c * S:(c + 1) * S].reshape(NVB, 1, VBS), 16, axis=1
        ).reshape(128, VBS)
        in_maps.append({
            "xTv": xTv,
            "disS": np.ascontiguousarray(disS),
            "w1pad": w1pad.reshape(NVB * 4, 128, 128),
            "selpad": selpad,
            "w2bd": w2bd,
            "bones": bones,
            "bcast8": bcast8,
            "b1rep": b1rep,
            "b2rep": b2rep,
            "gidx": gidx[c],
            "eidx": eidx[c],
        })

    res = bass_utils.run_bass_kernel_spmd(
        nc, in_maps, core_ids=list(range(C)), trace=_trace
    )

    out = np.empty((N, 7), np.float32)
    for c in range(C):
        o = res.results[c]["outF"].reshape(NVB, 16, VBS)
        out[c * S:(c + 1) * S] = o[:, 0:7, :].transpose(0, 2, 1).reshape(S, 7)
    if _trace or _debug:
        return out, res
    return out
